# revision 7
# baseline (speedup 1.0000x reference)
"""CoordCrossAtt Trainium2 kernel.

Full inputs -> full output. Internally: data-parallel over batch n across
8 NeuronCores (32 batches -> 4 per core); all params replicated.

Per batch (on device):
  x_h = mean_w(x)  [c,h] ;  x_w = mean_h(x)  [c,w]      (DVE reduces; /64
      folded into w_cv1 host-side)
  y   = w_cv1 @ [x_h|x_w] + b_cv1                        (PE, c contracted)
  q   = w_q @ y_h + b_q ; k = w_k @ y_w + b_k            (PE + ACT bias)
  vT  = (y_w)^T w_v^T + 1 (x) b_v                        (PE, bias via rank-1)
  a   = q^T k ; attn = softmax_w(SCALE * a)              (PE + DVE/ACT)
  z   = vT^T attn^T                                      (PE transpose + PE)
  ya  = sigmoid(w_proj @ z + b_proj)                     (PE + ACT)
  out = x * ya[c,h] (broadcast over w)                   (DVE, in-place)
"""

from contextlib import ExitStack

import numpy as np

import concourse.bacc as bacc
import concourse.bass as bass
import concourse.tile as tile
from concourse import bass_utils, mybir
from concourse.masks import make_identity

N_CORES = 8
N_FULL = 32          # full batch
NB = N_FULL // N_CORES  # batches per core
C = 512
H = 64
W = 64
MIP = 16
SCALE = (MIP // 1) ** -0.5  # heads=1
P = 128
CT = C // P          # channel tiles per batch
F32 = mybir.dt.float32

AFT = mybir.ActivationFunctionType
AX = mybir.AxisListType


def _bcast_w(ap2d, w):
    """[p, f] AP -> [p, f, w] AP broadcasting along a new innermost dim."""
    return bass.AP(
        tensor=ap2d.tensor,
        offset=ap2d.offset,
        ap=list(ap2d.ap) + [[0, w]],
    )


def build_module(debug=False):
    nc = bacc.Bacc("TRN2", target_bir_lowering=False, debug=debug)

    xs = nc.dram_tensor("xs", [NB, C, H, W], F32, kind="ExternalInput")
    wcv1t = nc.dram_tensor("wcv1t", [P, CT, MIP], F32, kind="ExternalInput")
    wqt = nc.dram_tensor("wqt", [MIP, MIP], F32, kind="ExternalInput")
    wkt = nc.dram_tensor("wkt", [MIP, MIP], F32, kind="ExternalInput")
    wvt = nc.dram_tensor("wvt", [MIP, MIP], F32, kind="ExternalInput")
    wpt = nc.dram_tensor("wpt", [MIP, C], F32, kind="ExternalInput")
    bcv1 = nc.dram_tensor("bcv1", [MIP, 1], F32, kind="ExternalInput")
    bq = nc.dram_tensor("bq", [MIP, 1], F32, kind="ExternalInput")
    bk = nc.dram_tensor("bk", [MIP, 1], F32, kind="ExternalInput")
    bv = nc.dram_tensor("bv", [W, MIP], F32, kind="ExternalInput")
    bpj = nc.dram_tensor("bpj", [P, CT], F32, kind="ExternalInput")
    out = nc.dram_tensor("out", [NB, C, H, W], F32, kind="ExternalOutput")

    with ExitStack() as ctx:
        tc = ctx.enter_context(tile.TileContext(nc))
        singles = ctx.enter_context(tc.tile_pool(name="singles", bufs=1))
        xpool = ctx.enter_context(tc.tile_pool(name="xpool", bufs=2 * CT))
        small = ctx.enter_context(tc.tile_pool(name="small", bufs=2))
        psum = ctx.enter_context(
            tc.tile_pool(name="psum", bufs=1, space=bass.MemorySpace.PSUM)
        )
        psum2 = ctx.enter_context(
            tc.tile_pool(name="psum2", bufs=2, space=bass.MemorySpace.PSUM)
        )

        # ---- constants (loaded once) ----
        wcv1_sb = singles.tile([P, CT, MIP], F32, tag="wcv1")
        nc.sync.dma_start(wcv1_sb[:], wcv1t[:])
        wqt_sb = singles.tile([MIP, MIP], F32, tag="wqt")
        nc.sync.dma_start(wqt_sb[:], wqt[:])
        wkt_sb = singles.tile([MIP, MIP], F32, tag="wkt")
        nc.sync.dma_start(wkt_sb[:], wkt[:])
        wvt_sb = singles.tile([MIP, MIP], F32, tag="wvt")
        nc.sync.dma_start(wvt_sb[:], wvt[:])
        wpt_sb = singles.tile([MIP, C], F32, tag="wpt")
        nc.sync.dma_start(wpt_sb[:], wpt[:])
        bcv1_sb = singles.tile([MIP, 1], F32, tag="bcv1")
        nc.sync.dma_start(bcv1_sb[:], bcv1[:])
        bq_sb = singles.tile([MIP, 1], F32, tag="bq")
        nc.sync.dma_start(bq_sb[:], bq[:])
        bk_sb = singles.tile([MIP, 1], F32, tag="bk")
        nc.sync.dma_start(bk_sb[:], bk[:])
        bv_sb = singles.tile([W, MIP], F32, tag="bv")
        nc.sync.dma_start(bv_sb[:], bv[:])
        bpj_sb = singles.tile([P, CT], F32, tag="bpj")
        nc.sync.dma_start(bpj_sb[:], bpj[:])
        ident = singles.tile([H, H], F32, tag="ident")
        make_identity(nc, ident[:])

        for n in range(NB):
            # ---- load x, pooled means ----
            x_tiles = []
            cat = small.tile([P, CT, H + W], F32, tag="cat")
            for ci in range(CT):
                xt = xpool.tile([P, H, W], F32, tag="x")
                nc.sync.dma_start(xt[:], xs[n, ci * P : (ci + 1) * P, :, :])
                x_tiles.append(xt)
                # sum over w -> [c,h] (mean /64 folded into w_cv1)
                nc.vector.reduce_sum(cat[:, ci, 0:H], xt[:], axis=AX.X)
                # sum over h -> [c,w]
                nc.vector.reduce_sum(
                    cat[:, ci, H : H + W], xt[:].transpose([0, 2, 1]), axis=AX.X
                )

            # ---- cv1: y[m, l] over l = [h | w] ----
            y_ps = psum.tile([MIP, H + W], F32, tag="y")
            for ci in range(CT):
                nc.tensor.matmul(
                    y_ps[:],
                    wcv1_sb[:, ci, :],
                    cat[:, ci, :],
                    start=(ci == 0),
                    stop=(ci == CT - 1),
                )
            y_sb = small.tile([MIP, H + W], F32, tag="ysb")
            nc.scalar.activation(
                y_sb[:], y_ps[:], AFT.Identity, bias=bcv1_sb[:], scale=1.0
            )

            # ---- q, k ----
            qk_ps = psum.tile([MIP, 2 * H], F32, tag="qk")
            nc.tensor.matmul(qk_ps[:, 0:H], wqt_sb[:], y_sb[:, 0:H], start=True, stop=True)
            nc.tensor.matmul(
                qk_ps[:, H : 2 * H], wkt_sb[:], y_sb[:, H : H + W], start=True, stop=True
            )
            q_sb = small.tile([MIP, H], F32, tag="qsb")
            nc.scalar.activation(q_sb[:], qk_ps[:, 0:H], AFT.Identity, bias=bq_sb[:], scale=1.0)
            k_sb = small.tile([MIP, W], F32, tag="ksb")
            nc.scalar.activation(
                k_sb[:], qk_ps[:, H : 2 * H], AFT.Identity, bias=bk_sb[:], scale=1.0
            )

            # ---- vT = yw^T @ wv^T, bias added via pre-broadcast [W, MIP] table ----
            vt_ps = psum.tile([W, MIP], F32, tag="vt")
            nc.tensor.matmul(
                vt_ps[:], y_sb[:, H : H + W], wvt_sb[:], start=True, stop=True
            )
            vt_sb = small.tile([W, MIP], F32, tag="vtsb")
            nc.vector.tensor_add(vt_sb[:], vt_ps[:], bv_sb[:])

            # ---- attention ----
            a_ps = psum.tile([H, W], F32, tag="a")
            nc.tensor.matmul(a_ps[:], q_sb[:], k_sb[:], start=True, stop=True)
            mx = small.tile([H, 1], F32, tag="mx")
            nc.vector.reduce_max(mx[:], a_ps[:], axis=AX.X)
            negm = small.tile([H, 1], F32, tag="negm")
            nc.vector.tensor_scalar_mul(negm[:], mx[:], -SCALE)
            e_sb = small.tile([H, W], F32, tag="esb")
            ssum = small.tile([H, 1], F32, tag="ssum")
            nc.scalar.activation(
                e_sb[:], a_ps[:], AFT.Exp, bias=negm[:], scale=SCALE, accum_out=ssum[:]
            )
            r_sb = small.tile([H, 1], F32, tag="rsb")
            nc.vector.reciprocal(r_sb[:], ssum[:])
            en_sb = small.tile([H, W], F32, tag="ensb")
            nc.scalar.mul(en_sb[:], e_sb[:], r_sb[:])

            # ---- z = vT^T @ attn^T ----
            et_ps = psum.tile([W, H], F32, tag="et")
            nc.tensor.transpose(et_ps[:], en_sb[:], ident[:])
            et_sb = small.tile([W, H], F32, tag="etsb")
            nc.scalar.copy(et_sb[:], et_ps[:])
            z_ps = psum.tile([MIP, H], F32, tag="z")
            nc.tensor.matmul(z_ps[:], vt_sb[:], et_sb[:], start=True, stop=True)
            z_sb = small.tile([MIP, H], F32, tag="zsb")
            nc.scalar.copy(z_sb[:], z_ps[:])

            # ---- proj + sigmoid + gate + store ----
            ya_sb = small.tile([P, CT, H], F32, tag="ya")
            for ci in range(CT):
                ya_ps = psum2.tile([P, H], F32, tag="yaps")
                nc.tensor.matmul(
                    ya_ps[:], wpt_sb[:, ci * P : (ci + 1) * P], z_sb[:],
                    start=True, stop=True,
                )
                nc.scalar.activation(
                    ya_sb[:, ci, :], ya_ps[:], AFT.Sigmoid,
                    bias=bpj_sb[:, ci : ci + 1], scale=1.0,
                )
                nc.vector.tensor_mul(
                    x_tiles[ci][:], x_tiles[ci][:], _bcast_w(ya_sb[:, ci, :], W)
                )
                nc.sync.dma_start(out[n, ci * P : (ci + 1) * P, :, :], x_tiles[ci][:])

    nc.compile()
    return nc


def prep_inputs(x_shard, w_cv1, b_cv1, w_q, b_q, w_k, b_k, w_v, b_v, w_proj, b_proj):
    """Host-side weight prep shared by all cores (tiny)."""
    f = np.float32
    # fold the 1/64 pooling mean into w_cv1; lhsT layout [c, m] -> [P, CT, MIP]
    wcv1t = np.ascontiguousarray((w_cv1 / np.float32(W)).T)  # [C, MIP]
    wcv1t = np.ascontiguousarray(wcv1t.reshape(CT, P, MIP).transpose(1, 0, 2))
    return {
        "xs": np.ascontiguousarray(x_shard, dtype=f),
        "wcv1t": wcv1t.astype(f),
        "wqt": np.ascontiguousarray(w_q.T, dtype=f),
        "wkt": np.ascontiguousarray(w_k.T, dtype=f),
        "wvt": np.ascontiguousarray(w_v.T, dtype=f),
        "wpt": np.ascontiguousarray(w_proj.T, dtype=f),
        "bcv1": np.ascontiguousarray(b_cv1.reshape(MIP, 1), dtype=f),
        "bq": np.ascontiguousarray(b_q.reshape(MIP, 1), dtype=f),
        "bk": np.ascontiguousarray(b_k.reshape(MIP, 1), dtype=f),
        "bv": np.ascontiguousarray(np.tile(b_v.reshape(1, MIP), (W, 1)), dtype=f),
        "bpj": np.ascontiguousarray(b_proj.reshape(CT, P).T, dtype=f),
    }


_module_cache = {}


def get_module(debug=False):
    key = bool(debug)
    if key not in _module_cache:
        _module_cache[key] = build_module(debug=debug)
    return _module_cache[key]


def kernel(x, w_cv1, b_cv1, w_q, b_q, w_k, b_k, w_v, b_v, w_proj, b_proj,
           trace=False):
    nc = get_module(debug=False)
    in_maps = []
    for i in range(N_CORES):
        shard = x[i * NB : (i + 1) * NB]
        in_maps.append(
            prep_inputs(shard, w_cv1, b_cv1, w_q, b_q, w_k, b_k, w_v, b_v,
                        w_proj, b_proj)
        )
    res = bass_utils.run_bass_kernel_spmd(
        nc, in_maps, core_ids=list(range(N_CORES)), trace=trace
    )
    outs = [res.results[i]["out"] for i in range(N_CORES)]
    full = np.concatenate(outs, axis=0).astype(np.float32)
    if trace:
        kernel.last_exec_time_ns = res.exec_time_ns
        kernel.last_results = res
    return full
